# revision 1
# baseline (speedup 1.0000x reference)
"""KoLeo-loss kernel for 8 Trainium2 NeuronCores.

Reference computation (for x of shape [B=16384, D=256] f32):
    xn   = x / ||x||_row                       (L2 row normalize)
    gram = xn @ xn.T
    min_dist_i = min_{j != i} sqrt(clip(2 - 2*gram_ij, 0))
    loss = -mean(log(min_dist + 1e-8))

Device strategy (one identical SPMD program on 8 cores):
  - Core c receives xr = roll(x, -c*2048, axis=0): its 2048 query rows are
    local rows 0..2047, and the self-match (diagonal) of local query m sits
    at local column m.  Row-max is permutation invariant, so rolling is free.
  - Phase A: load 128-row chunks, row-normalize in f32 (ACT square+accum,
    ACT sqrt, DVE reciprocal, DVE scale+cast to fp16), PE-transpose into a
    feature-major fp16 tile xT [128p(feature), 2(k), n_rows].
  - Phase B: for each 128-query chunk (stationary = slice of xT), stream all
    database columns through the PE in 512-col PSUM banks (K=256 as two
    accumulated passes).  Drain: ACT copies half the banks PSUM->SBUF f32;
    DVE tensor_tensor_reduce(max, max) consumes (psum bank, sbuf copy) pairs
    and maintains the running row max in a [128,1] accumulator.  Self-match
    is killed by adding -4 to the one 512-col bank holding the diagonal.
  - Output per core: gmax [128, 16] f32 (row-max of gram per query).
Host finishes: min_dist = sqrt(2-2*gmax), loss = -mean(log(min_dist+1e-8)).
"""

import sys

if "/opt/trn_rl_repo" not in sys.path:
    sys.path.insert(0, "/opt/trn_rl_repo")

import numpy as np

D = 256
P = 128
BANK = 512  # psum bank width in f32 elements
SPAN = 8  # psum banks in flight per span
B_FULL = 16384
N_CORES = 8
QPC = B_FULL // N_CORES  # queries per core


def make_dmask() -> np.ndarray:
    """dmask[p, t, j] = -4 where j == t*128+p else 0.

    Query chunk mc (local rows mc*128+p) has its self-match in bank mc//4
    at in-bank column (mc%4)*128 + p; tile t = mc%4 kills it.
    """
    dm = np.zeros((P, 4, BANK), dtype=np.float32)
    for t in range(4):
        for p in range(P):
            dm[p, t, t * P + p] = -4.0
    return dm


def build_nc(n_rows: int, n_q: int):
    import concourse.mybir as mybir
    import concourse.tile as tile
    from concourse import bacc
    from concourse.masks import make_identity

    dt = mybir.dt
    AF = mybir.ActivationFunctionType
    OP = mybir.AluOpType

    assert n_rows % (BANK * SPAN) == 0
    assert n_q % P == 0
    n_mc = n_q // P
    n_chunks = n_rows // P
    n_groups = n_chunks // 4
    n_banks = n_rows // BANK
    n_spans = n_banks // SPAN
    assert n_mc <= 4 * SPAN, "diag bank must land in span 0"

    nc = bacc.Bacc(None)
    x_in = nc.declare_dram_parameter("x", [n_rows, D], dt.float32, isOutput=False)
    dm_in = nc.declare_dram_parameter("dmask", [P, 4, BANK], dt.float32, isOutput=False)
    out_d = nc.declare_dram_parameter("gmax", [P, n_mc], dt.float32, isOutput=True)

    PAIR = 2 * BANK  # two psum banks per tile: fewer, bigger drain ops

    with tile.TileContext(nc) as tc:
        with (
            tc.tile_pool(name="persist", bufs=1) as persist,
            tc.tile_pool(name="ld", bufs=4) as ldp,
            tc.tile_pool(name="norm", bufs=6) as normp,
            tc.tile_pool(name="cp", bufs=8) as cpp,
            tc.tile_pool(name="mxp", bufs=2) as mxp,
            tc.tile_pool(name="ps", bufs=4, space="PSUM") as psp,
        ):
            xT = persist.tile([P, 2, n_rows], dt.float16)
            ident = persist.tile([P, P], dt.float16)
            make_identity(nc, ident)
            dmask = persist.tile([P, 4, BANK], dt.float32)
            nc.gpsimd.dma_start(out=dmask, in_=dm_in[:, :, :])
            gmax = persist.tile([P, n_mc], dt.float32)

            TRI = 3 * BANK
            QUAD = 3 * BANK  # macc width (банks folded per span position)

            # One span: 8 banks = TRI(0-2) + TRI(3-5) + PAIR(6-7) psum tiles.
            # ACT copies both TRIs to fp16 (6 banks, 2 ops @1423ns); DVE eats
            # the PAIR as a psum TT operand and folds all into macc [128,1536].
            def emit_span(mc, sp, macc):
                pt0 = psp.tile([P, TRI], dt.float32, tag="pst", bufs=2, name="pt0")
                pt1 = psp.tile([P, TRI], dt.float32, tag="pst", bufs=2, name="pt1")
                pt2 = psp.tile([P, PAIR], dt.float32, tag="psp", bufs=1, name="pt2")
                segs = [(pt0, 0, 3), (pt1, 3, 3), (pt2, 6, 2)]
                for k in range(2):
                    lhs = xT[:, k, mc * P : (mc + 1) * P]
                    b0 = sp * SPAN
                    for pt, off, nb in segs:
                        for h in range(nb):
                            nc.tensor.matmul(
                                pt[:, h * BANK : (h + 1) * BANK],
                                lhs,
                                xT[:, k, (b0 + off + h) * BANK : (b0 + off + h + 1) * BANK],
                                start=(k == 0),
                                stop=(k == 1),
                            )
                c0 = cpp.tile([P, TRI], dt.float16, tag="c0", bufs=3, name="c0")
                nc.scalar.copy(c0, pt0)
                c1 = cpp.tile([P, TRI], dt.float16, tag="c1", bufs=3, name="c1")
                nc.scalar.copy(c1, pt1)
                if sp == 0:
                    db = mc // 4  # diagonal bank 0..3: in c0 (0-2) or c1 (3)
                    src, off = (c0, db) if db < 3 else (c1, 0)
                    seg = src[:, off * BANK : (off + 1) * BANK]
                    nc.vector.tensor_tensor(seg, seg, dmask[:, mc % 4, :], OP.add)
                a = cpp.tile([P, PAIR], dt.float16, tag="a", bufs=3, name="a")
                nc.vector.tensor_tensor(a, pt2, c0[:, 0:PAIR], OP.max)
                b = cpp.tile([P, BANK], dt.float16, tag="b", bufs=3, name="b")
                nc.vector.tensor_tensor(
                    b, c0[:, PAIR:TRI], c1[:, PAIR:TRI], OP.max
                )
                if sp == 0:
                    nc.vector.tensor_tensor(
                        macc[:, 0:PAIR], c1[:, 0:PAIR], a, OP.max
                    )
                    nc.vector.tensor_copy(macc[:, PAIR:TRI], b)
                else:
                    c = cpp.tile([P, PAIR], dt.float16, tag="c", bufs=3, name="c")
                    nc.vector.tensor_tensor(c, c1[:, 0:PAIR], a, OP.max)
                    nc.vector.tensor_tensor(
                        macc[:, 0:PAIR], c, macc[:, 0:PAIR], OP.max
                    )
                    nc.vector.tensor_tensor(
                        macc[:, PAIR:TRI], b, macc[:, PAIR:TRI], OP.max
                    )

            def finish_mc(mc, macc):
                mh = cpp.tile([P, BANK], dt.float16, tag="mh", bufs=2, name="mh")
                nc.vector.tensor_tensor(
                    mh, macc[:, 0:BANK], macc[:, BANK:PAIR], OP.max
                )
                nc.vector.tensor_tensor(mh, macc[:, PAIR:TRI], mh, OP.max)
                nc.vector.tensor_reduce(
                    gmax[:, mc : mc + 1], mh, axis=mybir.AxisListType.X, op=OP.max
                )

            # ---------------- PE warmup burst (HAM un-throttle) -------------
            wps = psp.tile([P, P], dt.float32, tag="pst", bufs=2, name="warm")
            for _ in range(24):
                nc.tensor.matmul(wps, ident, ident, start=True, stop=True)

            # ---------------- Phase A: normalize + transpose ----------------
            # mc=0's spans are interleaved: span sp only needs banks
            # 8sp..8sp+7 = groups 8sp..8sp+7, so it runs as soon as they land.
            macc0 = mxp.tile([P, QUAD], dt.float16, tag="macc", name="macc0")
            xv = x_in[:, :].rearrange("(g c p) d -> g p c d", c=4, p=P)
            for g in range(n_groups):
                xa = ldp.tile([P, 4, D], dt.float32, tag="xa")
                nc.gpsimd.dma_start(out=xa, in_=xv[g])
                n2 = normp.tile([P, 4], dt.float32, tag="n2")
                sq = normp.tile([P, D], dt.float16, tag="sq")
                for c in range(4):
                    nc.scalar.activation(
                        out=sq,
                        in_=xa[:, c, :],
                        func=AF.Square,
                        accum_out=n2[:, c : c + 1],
                    )
                nrm = normp.tile([P, 4], dt.float32, tag="nrm")
                nc.scalar.sqrt(nrm, n2)
                rn = normp.tile([P, 4], dt.float32, tag="rn")
                nc.vector.reciprocal(rn, nrm)
                xn = normp.tile([P, 4, D], dt.float16, tag="xn")
                for c in range(4):
                    nc.vector.tensor_scalar_mul(
                        xn[:, c, :], xa[:, c, :], rn[:, c : c + 1]
                    )
                # Transpose via NORMAL matmul (out = xn_half.T @ I): faster
                # than transpose-mode and counts as PE activity for HAM.
                # Two chunks share one psum tile so the drain copy runs FD=512.
                for cc in range(2):
                    pst = psp.tile([P, 2, 2 * P], dt.float32, tag="pst", bufs=2)
                    for ci in range(2):
                        c = 2 * cc + ci
                        for k in range(2):
                            nc.tensor.matmul(
                                pst[:, k, ci * P : (ci + 1) * P],
                                xn[:, c, k * P : (k + 1) * P],
                                ident,
                                start=True,
                                stop=True,
                            )
                    s = g * 4 + 2 * cc
                    dst = xT[:, :, s * P : (s + 2) * P]
                    nc.vector.tensor_copy(dst, pst)
                if g % 8 == 7 and (g // 8) < n_spans:
                    emit_span(0, g // 8, macc0)
            finish_mc(0, macc0)

            # ---------------- Phase B: remaining query chunks ---------------
            for mc in range(1, n_mc):
                macc = mxp.tile([P, QUAD], dt.float16, tag="macc")
                for sp in range(n_spans):
                    emit_span(mc, sp, macc)
                finish_mc(mc, macc)

            nc.sync.dma_start(out=out_d[:, :], in_=gmax)

    nc.compile()
    return nc


_NC_CACHE = {}


def _get_nc(n_rows, n_q):
    key = (n_rows, n_q)
    if key not in _NC_CACHE:
        _NC_CACHE[key] = build_nc(n_rows, n_q)
    return _NC_CACHE[key]


LAST_RESULT = None  # BassKernelResults of the most recent run (for profiling)


def kernel(student_output: np.ndarray) -> np.ndarray:
    import os

    from concourse.bass_utils import run_bass_kernel_spmd

    global LAST_RESULT
    x = np.ascontiguousarray(student_output, dtype=np.float32)
    assert x.shape == (B_FULL, D)

    nc = _get_nc(B_FULL, QPC)
    dm = make_dmask()
    in_maps = [
        {"x": np.roll(x, -c * QPC, axis=0), "dmask": dm} for c in range(N_CORES)
    ]
    trace = bool(int(os.environ.get("KOLEO_TRACE", "0")))
    res = run_bass_kernel_spmd(
        nc, in_maps, core_ids=list(range(N_CORES)), trace=trace
    )
    LAST_RESULT = res

    gmax = np.empty(B_FULL, dtype=np.float32)
    for c in range(N_CORES):
        gm = res.results[c]["gmax"]  # [128, n_mc]
        gmax[c * QPC : (c + 1) * QPC] = gm.T.ravel()

    min_dist = np.sqrt(np.clip(2.0 - 2.0 * gmax.astype(np.float64), 0.0, None))
    loss = -np.mean(np.log(min_dist + 1e-8))
    return np.float32(loss)


if __name__ == "__main__":
    rng = np.random.default_rng(0)
    x = rng.standard_normal((B_FULL, D), dtype=np.float32)
    out = kernel(x)
    print("loss:", out)



# revision 2
# speedup vs baseline: 1.1033x; 1.1033x over previous
"""KoLeo-loss kernel v2 for 8 Trainium2 NeuronCores.

Reference computation (x of shape [B=16384, D=256] f32):
    xn   = x / ||x||_row
    gram = xn @ xn.T
    min_dist_i = min_{j != i} sqrt(clip(2 - 2*gram_ij, 0))
    loss = -mean(log(min_dist + 1e-8))

Key ideas vs the naive full-gram row-max kernel:
  1. Host does normalize + transpose: device receives xq[p, k, j] =
     fp16(16 * xn[j, 128k+p]) -- a feature-major operand both matmul sides
     share. No on-device phase A at all.
  2. Sharp log-sum-exp replaces the row max: with S_i = sum_j exp(B*g_ij),
     max_j g_ij ~= ln(S_i)/B for B = 768.  The row reduction then rides for
     free on the ACT engine's exp pass (accum_out), and the column-direction
     reduction becomes a *sum*, which is cheap elementwise DVE adds + a host
     partition-sum.  Total per-gram-element engine cost ~1 ACT cycle.
  3. Circulant half coverage: core c takes query rows [2048c, 2048c+2048)
     and database columns [2048c, 2048c+10240) (self block + next 4 blocks,
     mod B).  Every unordered pair is covered >= 1x; block-distance-4 pairs
     and some intra-block pairs are covered 2x, which only inflates the LSE
     by ln(2)/B ~= 0.0009 in gram units -- far below tolerance.  The self
     block is covered triangularly (row chunk qi spans cols [512*(qi//4),
     2048)) to save work.  This makes all 8 cores run one identical SPMD
     program on column-rolled inputs.

Device per span (one query chunk x up to 2048 db cols):
    PE   : 8 matmuls fp16 [128,512] K=2x128 -> psum f32 (= 256 * gram)
    DVE  : diag kill (self block only): += -1e9 identity segment
    ACT  : exp(scale*psum + bias) -> bf16 tile, accum_out -> row partial
    DVE  : S_blk += exp tile  (bf16 elementwise)
Outputs: srow [128, 80] f32 (row partials per (blk, qi)), scol [128,5,2048]
bf16 (column partial sums, partition dim unsummed).  Host: assemble S_i,
gmax = (ln S + BIAS)/BETA, min_dist = sqrt(2-2 gmax), loss.
"""

import sys

if "/opt/trn_rl_repo" not in sys.path:
    sys.path.insert(0, "/opt/trn_rl_repo")

import numpy as np

P = 128
D = 256
B_FULL = 16384
N_CORES = 8
BLK = 2048  # db block width = query rows per core
NBLK = 5  # db blocks per core (self + 4 ahead)
NQ = 16  # query chunks per core

BETA = 768.0
BIAS = 227.0
SCALE_IN = 16.0  # host premultiplier; psum = SCALE_IN^2 * gram
ACT_SCALE = BETA / (SCALE_IN * SCALE_IN)  # exp(ACT_SCALE*psum - BIAS)


def build_nc():
    import concourse.mybir as mybir
    import concourse.tile as tile
    from concourse import bacc
    from concourse.masks import make_identity

    dt = mybir.dt
    AF = mybir.ActivationFunctionType
    OP = mybir.AluOpType

    nc = bacc.Bacc(None)
    xq_in = nc.declare_dram_parameter("xq", [P, 2, B_FULL], dt.float16, isOutput=False)
    dm_in = nc.declare_dram_parameter("dmask", [P, P], dt.float32, isOutput=False)
    srow_d = nc.declare_dram_parameter("srow", [P, NBLK * NQ], dt.float32, isOutput=True)
    scol_d = nc.declare_dram_parameter("scol", [P, NBLK, BLK], dt.bfloat16, isOutput=True)

    with tile.TileContext(nc) as tc:
        with (
            tc.tile_pool(name="persist", bufs=1) as persist,
            tc.tile_pool(name="ep", bufs=3) as ep,
            tc.tile_pool(name="sp", bufs=2) as sp,
            tc.tile_pool(name="ps", bufs=2, space="PSUM") as psp,
        ):
            xT = persist.tile([P, 2, B_FULL], dt.float16)
            dmask = persist.tile([P, P], dt.float32)
            nc.sync.dma_start(out=dmask, in_=dm_in[:, :])
            srow_sb = persist.tile([P, NBLK * NQ], dt.float32)
            biasap = persist.tile([P, 1], dt.float32)
            nc.vector.memset(biasap, -BIAS)
            ident = persist.tile([P, P], dt.float16)
            make_identity(nc, ident)

            # stream the db operand in block chunks (overlaps with compute)
            for blk in range(NBLK):
                nc.gpsimd.dma_start(
                    out=xT[:, :, blk * BLK : (blk + 1) * BLK],
                    in_=xq_in[:, :, blk * BLK : (blk + 1) * BLK],
                )

            # HAM warmup: keep PE busy while the first DMA chunk lands.
            wps = psp.tile([P, BLK], dt.float32, tag="ps", name="warm")
            for _ in range(32):
                nc.tensor.matmul(wps[:, :P], ident, ident, start=True, stop=True)

            for blk in range(NBLK):
                sblk = sp.tile([P, BLK], dt.bfloat16, tag="s")
                for qi in range(NQ):
                    c0 = 512 * (qi // 4) if blk == 0 else 0
                    ps = psp.tile([P, BLK], dt.float32, tag="ps")
                    for k in range(2):
                        lhs = xT[:, k, qi * P : (qi + 1) * P]
                        for s in range(c0 // 512, 4):
                            col = blk * BLK + s * 512
                            nc.tensor.matmul(
                                ps[:, s * 512 : (s + 1) * 512],
                                lhs,
                                xT[:, k, col : col + 512],
                                start=(k == 0),
                                stop=(k == 1),
                            )
                    if blk == 0:
                        seg = ps[:, qi * P : (qi + 1) * P]
                        nc.vector.tensor_tensor(seg, seg, dmask, OP.add)
                    e = ep.tile([P, BLK], dt.bfloat16, tag="e")
                    nc.scalar.activation(
                        out=e[:, c0:BLK],
                        in_=ps[:, c0:BLK],
                        func=AF.Exp,
                        bias=biasap,
                        scale=ACT_SCALE,
                        accum_out=srow_sb[:, blk * NQ + qi : blk * NQ + qi + 1],
                    )
                    if qi == 0:
                        nc.vector.tensor_copy(sblk, e)
                    else:
                        nc.vector.tensor_tensor(
                            sblk[:, c0:BLK], e[:, c0:BLK], sblk[:, c0:BLK], OP.add
                        )
                nc.sync.dma_start(out=scol_d[:, blk, :], in_=sblk)

            nc.sync.dma_start(out=srow_d[:, :], in_=srow_sb)

    nc.compile()
    return nc


_NC_CACHE = {}


def _get_nc():
    if "nc" not in _NC_CACHE:
        _NC_CACHE["nc"] = build_nc()
    return _NC_CACHE["nc"]


LAST_RESULT = None


def kernel(student_output: np.ndarray) -> np.ndarray:
    import os

    from concourse.bass_utils import run_bass_kernel_spmd

    global LAST_RESULT
    x = np.ascontiguousarray(student_output, dtype=np.float32)
    assert x.shape == (B_FULL, D)

    norm = np.maximum(np.linalg.norm(x, axis=1, keepdims=True), 1e-12)
    xn = x / norm
    xq = (xn * SCALE_IN).astype(np.float16)  # [B, 256]
    # xT[p, k, j] = xq[j, 128k+p]
    xT = np.ascontiguousarray(xq.T.reshape(2, P, B_FULL).transpose(1, 0, 2))

    dm = np.zeros((P, P), np.float32)
    np.fill_diagonal(dm, -1.0e9)

    nc = _get_nc()
    in_maps = [
        {"xq": np.roll(xT, -c * BLK, axis=2), "dmask": dm} for c in range(N_CORES)
    ]
    trace = bool(int(os.environ.get("KOLEO_TRACE", "0")))
    res = run_bass_kernel_spmd(
        nc, in_maps, core_ids=list(range(N_CORES)), trace=trace
    )
    LAST_RESULT = res

    S = np.zeros(B_FULL, np.float64)
    for c in range(N_CORES):
        srow = np.asarray(res.results[c]["srow"], dtype=np.float64)  # [128, 80]
        scol = np.asarray(
            res.results[c]["scol"].astype(np.float32), dtype=np.float64
        )  # [128, 5, 2048]
        rp = srow.reshape(P, NBLK, NQ).sum(axis=1)  # [p, qi]
        rloc = rp.T.reshape(BLK)  # local row qi*128+p
        cloc = scol.sum(axis=0).reshape(NBLK * BLK)  # local col j
        base = c * BLK
        idx = (np.arange(BLK) + base) % B_FULL
        S[idx] += rloc
        for blk in range(NBLK):
            j0 = (base + blk * BLK) % B_FULL
            S[j0 : j0 + BLK] += cloc[blk * BLK : (blk + 1) * BLK]

    est = (np.log(S) + BIAS) / BETA  # smooth-max of scaled gram
    md = np.sqrt(np.clip(2.0 - 2.0 * est, 0.0, None))
    loss = -np.mean(np.log(md + 1e-8))
    return np.float32(loss)


if __name__ == "__main__":
    rng = np.random.default_rng(0)
    x = rng.standard_normal((B_FULL, D), dtype=np.float32)
    out = kernel(x)
    print("loss:", out)


# revision 3
# speedup vs baseline: 1.1276x; 1.0221x over previous
"""KoLeo-loss kernel v3 for 8 Trainium2 NeuronCores.

v2 -> v3:
  - fp8 e4m3 inputs (host: 16*xn) with DoubleRow matmuls: K=256 in one PE
    pass at 2x rate.  Gram quantization error measured host-side: 2.7e-4
    rel on the final loss (tolerance 2e-2).
  - Block 4 (block-distance-4 pairs) is covered by BOTH endpoint cores, so
    each core only needs the ROW direction there: one DVE tensor_reduce
    (max) per span replaces exp + column sums.  Host combines
    max(LSE(blocks 0-3), blk4_rowmax).
  - Self block triangular at 128-col granularity.
  - blk4 spans are interleaved between LSE spans so PE/ACT/DVE overlap.

See kernel_v2.py docstring for the overall scheme (circulant cover, LSE).
"""

import sys

if "/opt/trn_rl_repo" not in sys.path:
    sys.path.insert(0, "/opt/trn_rl_repo")

import numpy as np

P = 128
D = 256
B_FULL = 16384
N_CORES = 8
BLK = 2048
NLSE = 4  # blocks 0..3 via LSE
NQ = 16

BETA = 768.0
BIAS = 227.0
SCALE_IN = 16.0
ACT_SCALE = BETA / (SCALE_IN * SCALE_IN)


def build_nc():
    import concourse.mybir as mybir
    import concourse.tile as tile
    from concourse import bacc
    from concourse.masks import make_identity

    dt = mybir.dt
    AF = mybir.ActivationFunctionType
    OP = mybir.AluOpType
    DR = mybir.MatmulPerfMode.DoubleRow

    nc = bacc.Bacc(None)
    xq_in = nc.declare_dram_parameter("xq", [P, 2, B_FULL], dt.float8e4, isOutput=False)
    dm_in = nc.declare_dram_parameter("dmask", [P, P], dt.float32, isOutput=False)
    srow_d = nc.declare_dram_parameter("srow", [P, NLSE * NQ], dt.float32, isOutput=True)
    scol_d = nc.declare_dram_parameter("scol", [P, NLSE, BLK], dt.bfloat16, isOutput=True)
    rmax_d = nc.declare_dram_parameter("rmax", [P, NQ], dt.float32, isOutput=True)

    def emit_mms(ps, xT, qi, blk, c0):
        """Gram matmuls for span (qi, blk) covering span cols [c0, 2048)."""
        lhs = xT[:, :, qi * P : (qi + 1) * P]
        c = c0
        while c < BLK:
            w = min(512 - (c % 512), BLK - c)
            col = blk * BLK + c
            nc.tensor.matmul(
                ps[:, c : c + w],
                lhs,
                xT[:, :, col : col + w],
                start=True,
                stop=True,
                perf_mode=DR,
            )
            c += w

    with tile.TileContext(nc) as tc:
        with (
            tc.tile_pool(name="persist", bufs=1) as persist,
            tc.tile_pool(name="ep", bufs=3) as ep,
            tc.tile_pool(name="sp", bufs=2) as sp,
            tc.tile_pool(name="ps", bufs=2, space="PSUM") as psp,
        ):
            xT = persist.tile([P, 2, B_FULL], dt.float8e4)
            dmask = persist.tile([P, P], dt.float32)
            nc.sync.dma_start(out=dmask, in_=dm_in[:, :])
            srow_sb = persist.tile([P, NLSE * NQ], dt.float32)
            rmax_sb = persist.tile([P, NQ], dt.float32)
            biasap = persist.tile([P, 1], dt.float32)
            nc.vector.memset(biasap, -BIAS)
            ident = persist.tile([P, P], dt.float16)
            make_identity(nc, ident)
            # preload the exp table set while DMA streams
            scratch1 = persist.tile([P, 1], dt.float32)
            nc.scalar.activation(
                out=scratch1, in_=biasap, func=AF.Exp, bias=biasap, scale=0.0
            )

            for blk in range(5):
                nc.gpsimd.dma_start(
                    out=xT[:, :, blk * BLK : (blk + 1) * BLK],
                    in_=xq_in[:, :, blk * BLK : (blk + 1) * BLK],
                )

            wps = psp.tile([P, BLK], dt.float32, tag="ps", name="warm")
            for _ in range(32):
                nc.tensor.matmul(wps[:, :P], ident, ident, start=True, stop=True)

            def emit_blk4(qi):
                ps = psp.tile([P, BLK], dt.float32, tag="ps")
                emit_mms(ps, xT, qi, 4, 0)
                nc.vector.tensor_reduce(
                    rmax_sb[:, qi : qi + 1], ps, axis=mybir.AxisListType.X, op=OP.max
                )

            # Process full-width blocks 1..3 first (their DMA chunks land in
            # order and the spans are uniform), the ragged self block last.
            # blk4 row-max spans interleave 2-per-4 starting at it=16, by
            # which time the blk4 DMA chunk has landed.
            it = 0
            for blk in (1, 2, 3, 0):
                sblk = sp.tile([P, BLK], dt.bfloat16, tag="s")
                for qi in range(NQ):
                    c0 = P * qi if blk == 0 else 0
                    ps = psp.tile([P, BLK], dt.float32, tag="ps")
                    emit_mms(ps, xT, qi, blk, c0)
                    if blk == 0:
                        seg = ps[:, qi * P : (qi + 1) * P]
                        nc.vector.tensor_tensor(seg, seg, dmask, OP.add)
                    e = ep.tile([P, BLK], dt.bfloat16, tag="e")
                    nc.scalar.activation(
                        out=e[:, c0:BLK],
                        in_=ps[:, c0:BLK],
                        func=AF.Exp,
                        bias=biasap,
                        scale=ACT_SCALE,
                        accum_out=srow_sb[:, blk * NQ + qi : blk * NQ + qi + 1],
                    )
                    if qi == 0:
                        nc.vector.tensor_copy(sblk, e)
                    elif blk == 0:
                        nc.vector.tensor_tensor(
                            sblk[:, c0:BLK], e[:, c0:BLK], sblk[:, c0:BLK], OP.add
                        )
                    else:
                        nc.vector.tensor_tensor(sblk, e, sblk, OP.add)
                    if 16 <= it < 48 and it % 2 == 1:
                        emit_blk4((it - 16) // 2)
                    it += 1
                nc.sync.dma_start(out=scol_d[:, blk, :], in_=sblk)

            nc.sync.dma_start(out=srow_d[:, :], in_=srow_sb)
            nc.sync.dma_start(out=rmax_d[:, :], in_=rmax_sb)

    nc.compile()
    return nc


_NC_CACHE = {}


def _get_nc():
    if "nc" not in _NC_CACHE:
        _NC_CACHE["nc"] = build_nc()
    return _NC_CACHE["nc"]


LAST_RESULT = None


def kernel(student_output: np.ndarray) -> np.ndarray:
    import os

    import ml_dtypes
    from concourse.bass_utils import run_bass_kernel_spmd

    global LAST_RESULT
    x = np.ascontiguousarray(student_output, dtype=np.float32)
    assert x.shape == (B_FULL, D)

    norm = np.maximum(np.linalg.norm(x, axis=1, keepdims=True), 1e-12)
    xn = x / norm
    xq = (xn * SCALE_IN).astype(ml_dtypes.float8_e4m3)  # [B, 256]
    xT = np.ascontiguousarray(xq.T.reshape(2, P, B_FULL).transpose(1, 0, 2))

    dm = np.zeros((P, P), np.float32)
    np.fill_diagonal(dm, -1.0e9)

    nc = _get_nc()
    in_maps = [
        {"xq": np.roll(xT, -c * BLK, axis=2), "dmask": dm} for c in range(N_CORES)
    ]
    trace = bool(int(os.environ.get("KOLEO_TRACE", "0")))
    res = run_bass_kernel_spmd(
        nc, in_maps, core_ids=list(range(N_CORES)), trace=trace
    )
    LAST_RESULT = res

    S = np.zeros(B_FULL, np.float64)
    gmax4 = np.zeros(B_FULL, np.float64)
    for c in range(N_CORES):
        srow = np.asarray(res.results[c]["srow"], dtype=np.float64)  # [128, 64]
        scol = np.asarray(
            res.results[c]["scol"].astype(np.float32), dtype=np.float64
        )  # [128, 4, 2048]
        rmax = np.asarray(res.results[c]["rmax"], dtype=np.float64)  # [128, 16]
        rp = srow.reshape(P, NLSE, NQ).sum(axis=1)  # [p, qi]
        rloc = rp.T.reshape(BLK)
        cloc = scol.sum(axis=0).reshape(NLSE * BLK)
        mloc = rmax.T.reshape(BLK)  # local row qi*128+p
        base = c * BLK
        S[base : base + BLK] += rloc
        gmax4[base : base + BLK] = mloc / (SCALE_IN * SCALE_IN)
        for blk in range(NLSE):
            j0 = (base + blk * BLK) % B_FULL
            S[j0 : j0 + BLK] += cloc[blk * BLK : (blk + 1) * BLK]

    est = (np.log(S) + BIAS) / BETA
    g = np.maximum(est, gmax4)
    md = np.sqrt(np.clip(2.0 - 2.0 * g, 0.0, None))
    loss = -np.mean(np.log(md + 1e-8))
    return np.float32(loss)


if __name__ == "__main__":
    rng = np.random.default_rng(0)
    x = rng.standard_normal((B_FULL, D), dtype=np.float32)
    out = kernel(x)
    print("loss:", out)


# revision 4
# speedup vs baseline: 1.1385x; 1.0097x over previous
"""KoLeo-loss kernel v3 for 8 Trainium2 NeuronCores.

v2 -> v3:
  - fp8 e4m3 inputs (host: 16*xn) with DoubleRow matmuls: K=256 in one PE
    pass at 2x rate.  Gram quantization error measured host-side: 2.7e-4
    rel on the final loss (tolerance 2e-2).
  - Block 4 (block-distance-4 pairs) is covered by BOTH endpoint cores, so
    each core only needs the ROW direction there: one DVE tensor_reduce
    (max) per span replaces exp + column sums.  Host combines
    max(LSE(blocks 0-3), blk4_rowmax).
  - Self block triangular at 128-col granularity.
  - blk4 spans are interleaved between LSE spans so PE/ACT/DVE overlap.

See kernel_v2.py docstring for the overall scheme (circulant cover, LSE).
"""

import sys

if "/opt/trn_rl_repo" not in sys.path:
    sys.path.insert(0, "/opt/trn_rl_repo")

import numpy as np

P = 128
D = 256
B_FULL = 16384
N_CORES = 8
BLK = 2048
NLSE = 4  # blocks 0..3 via LSE
NQ = 16

BETA = 768.0
BIAS = 227.0
SCALE_IN = 16.0
ACT_SCALE = BETA / (SCALE_IN * SCALE_IN)


def build_nc():
    import concourse.mybir as mybir
    import concourse.tile as tile
    from concourse import bacc
    from concourse.masks import make_identity

    dt = mybir.dt
    AF = mybir.ActivationFunctionType
    OP = mybir.AluOpType
    DR = mybir.MatmulPerfMode.DoubleRow

    nc = bacc.Bacc(None)
    xq_in = nc.declare_dram_parameter("xq", [P, 2, B_FULL], dt.float8e4, isOutput=False)
    dm_in = nc.declare_dram_parameter("dmask", [P, P], dt.float32, isOutput=False)
    srow_d = nc.declare_dram_parameter("srow", [P, NLSE * NQ], dt.float32, isOutput=True)
    # two column-sum accumulators per block: [0]=DVE chain, [1]=GPSIMD chain
    scol_d = nc.declare_dram_parameter(
        "scol", [P, NLSE, 2, BLK], dt.bfloat16, isOutput=True
    )
    rmax_d = nc.declare_dram_parameter("rmax", [P, NQ], dt.float32, isOutput=True)

    def emit_mms(ps, xT, qi, blk, c0):
        """Gram matmuls for span (qi, blk) covering span cols [c0, 2048)."""
        lhs = xT[:, :, qi * P : (qi + 1) * P]
        c = c0
        while c < BLK:
            w = min(512 - (c % 512), BLK - c)
            col = blk * BLK + c
            nc.tensor.matmul(
                ps[:, c : c + w],
                lhs,
                xT[:, :, col : col + w],
                start=True,
                stop=True,
                perf_mode=DR,
            )
            c += w

    with tile.TileContext(nc) as tc:
        with (
            tc.tile_pool(name="persist", bufs=1) as persist,
            tc.tile_pool(name="ep", bufs=4) as ep,
            tc.tile_pool(name="sp", bufs=2) as sp,
            tc.tile_pool(name="ps", bufs=2, space="PSUM") as psp,
        ):
            xT = persist.tile([P, 2, B_FULL], dt.float8e4)
            dmask = persist.tile([P, P], dt.float32)
            nc.sync.dma_start(out=dmask, in_=dm_in[:, :])
            srow_sb = persist.tile([P, NLSE * NQ], dt.float32)
            rmax_sb = persist.tile([P, NQ], dt.float32)
            biasap = persist.tile([P, 1], dt.float32)
            nc.vector.memset(biasap, -BIAS)
            ident = persist.tile([P, P], dt.float16)
            make_identity(nc, ident)
            # preload the exp table set while DMA streams
            scratch1 = persist.tile([P, 1], dt.float32)
            nc.scalar.activation(
                out=scratch1, in_=biasap, func=AF.Exp, bias=biasap, scale=0.0
            )

            # alternate two DMA queues so chunks stream concurrently and the
            # first chunk (block 0 -- processed first) lands asap
            for i, ch in enumerate(range(0, 5 * BLK, BLK // 2)):
                q = nc.gpsimd if i % 2 == 0 else nc.sync
                q.dma_start(
                    out=xT[:, :, ch : ch + BLK // 2],
                    in_=xq_in[:, :, ch : ch + BLK // 2],
                )

            wps = psp.tile([P, BLK], dt.float32, tag="ps", name="warm")
            for _ in range(32):
                nc.tensor.matmul(wps[:, :P], ident, ident, start=True, stop=True)

            def emit_blk4(qi):
                ps = psp.tile([P, BLK], dt.float32, tag="ps")
                emit_mms(ps, xT, qi, 4, 0)
                nc.vector.tensor_reduce(
                    rmax_sb[:, qi : qi + 1], ps, axis=mybir.AxisListType.X, op=OP.max
                )

            # Self block first (needs only DMA chunk 0), with qi descending so
            # the narrow triangular spans fill the DMA window.  Then blocks
            # 1..3 full width; blk4 row-max spans interleave one per three
            # LSE spans once its DMA chunk is resident.  A slice of the
            # column-sum accumulation goes to GPSIMD (own chain) to keep DVE
            # below ACT.
            GP_QI = ()
            it = 0
            for blk in (0, 1, 2, 3):
                sblk = sp.tile([P, BLK], dt.bfloat16, tag="s")
                sblk_g = None
                qis = range(NQ - 1, -1, -1) if blk == 0 else range(NQ)
                if blk == 0:
                    nc.vector.memset(sblk, 0.0)
                for qi in qis:
                    c0 = P * qi if blk == 0 else 0
                    ps = psp.tile([P, BLK], dt.float32, tag="ps")
                    emit_mms(ps, xT, qi, blk, c0)
                    if blk == 0:
                        seg = ps[:, qi * P : (qi + 1) * P]
                        nc.vector.tensor_tensor(seg, seg, dmask, OP.add)
                    e = ep.tile([P, BLK], dt.bfloat16, tag="e")
                    nc.scalar.activation(
                        out=e[:, c0:BLK],
                        in_=ps[:, c0:BLK],
                        func=AF.Exp,
                        bias=biasap,
                        scale=ACT_SCALE,
                        accum_out=srow_sb[:, blk * NQ + qi : blk * NQ + qi + 1],
                    )
                    if blk == 0:
                        nc.vector.tensor_tensor(
                            sblk[:, c0:BLK], e[:, c0:BLK], sblk[:, c0:BLK], OP.add
                        )
                    elif qi in GP_QI:
                        if sblk_g is None:
                            sblk_g = sp.tile([P, BLK], dt.bfloat16, tag="sg")
                            nc.gpsimd.tensor_copy(sblk_g, e)
                        else:
                            nc.gpsimd.tensor_tensor(sblk_g, e, sblk_g, OP.add)
                    elif qi == 0:
                        nc.vector.tensor_copy(sblk, e)
                    else:
                        nc.vector.tensor_tensor(sblk, e, sblk, OP.add)
                    if it >= 16 and (it - 16) % 3 == 0:
                        emit_blk4((it - 16) // 3)
                    it += 1
                nc.sync.dma_start(out=scol_d[:, blk, 0, :], in_=sblk)
                if sblk_g is not None:
                    nc.sync.dma_start(out=scol_d[:, blk, 1, :], in_=sblk_g)

            nc.sync.dma_start(out=srow_d[:, :], in_=srow_sb)
            nc.sync.dma_start(out=rmax_d[:, :], in_=rmax_sb)

    nc.compile()
    return nc


_NC_CACHE = {}


def _get_nc():
    if "nc" not in _NC_CACHE:
        _NC_CACHE["nc"] = build_nc()
    return _NC_CACHE["nc"]


LAST_RESULT = None


def kernel(student_output: np.ndarray) -> np.ndarray:
    import os

    import ml_dtypes
    from concourse.bass_utils import run_bass_kernel_spmd

    global LAST_RESULT
    x = np.ascontiguousarray(student_output, dtype=np.float32)
    assert x.shape == (B_FULL, D)

    norm = np.maximum(np.linalg.norm(x, axis=1, keepdims=True), 1e-12)
    xn = x / norm
    xq = (xn * SCALE_IN).astype(ml_dtypes.float8_e4m3)  # [B, 256]
    xT = np.ascontiguousarray(xq.T.reshape(2, P, B_FULL).transpose(1, 0, 2))

    dm = np.zeros((P, P), np.float32)
    np.fill_diagonal(dm, -1.0e9)

    nc = _get_nc()
    in_maps = [
        {"xq": np.roll(xT, -c * BLK, axis=2), "dmask": dm} for c in range(N_CORES)
    ]
    trace = bool(int(os.environ.get("KOLEO_TRACE", "0")))
    res = run_bass_kernel_spmd(
        nc, in_maps, core_ids=list(range(N_CORES)), trace=trace
    )
    LAST_RESULT = res

    S = np.zeros(B_FULL, np.float64)
    gmax4 = np.zeros(B_FULL, np.float64)
    for c in range(N_CORES):
        srow = np.asarray(res.results[c]["srow"], dtype=np.float64)  # [128, 64]
        scol = np.asarray(
            res.results[c]["scol"].astype(np.float32), dtype=np.float64
        )  # [128, 4, 2, 2048]
        rmax = np.asarray(res.results[c]["rmax"], dtype=np.float64)  # [128, 16]
        rp = srow.reshape(P, NLSE, NQ).sum(axis=1)  # [p, qi]
        rloc = rp.T.reshape(BLK)
        cloc = scol.sum(axis=(0, 2)).reshape(NLSE * BLK)
        mloc = rmax.T.reshape(BLK)  # local row qi*128+p
        base = c * BLK
        S[base : base + BLK] += rloc
        gmax4[base : base + BLK] = mloc / (SCALE_IN * SCALE_IN)
        for blk in range(NLSE):
            j0 = (base + blk * BLK) % B_FULL
            S[j0 : j0 + BLK] += cloc[blk * BLK : (blk + 1) * BLK]

    est = (np.log(S) + BIAS) / BETA
    g = np.maximum(est, gmax4)
    md = np.sqrt(np.clip(2.0 - 2.0 * g, 0.0, None))
    loss = -np.mean(np.log(md + 1e-8))
    return np.float32(loss)


if __name__ == "__main__":
    rng = np.random.default_rng(0)
    x = rng.standard_normal((B_FULL, D), dtype=np.float32)
    out = kernel(x)
    print("loss:", out)


# revision 5
# speedup vs baseline: 1.1413x; 1.0024x over previous
"""KoLeo-loss kernel v3 for 8 Trainium2 NeuronCores.

v2 -> v3:
  - fp8 e4m3 inputs (host: 16*xn) with DoubleRow matmuls: K=256 in one PE
    pass at 2x rate.  Gram quantization error measured host-side: 2.7e-4
    rel on the final loss (tolerance 2e-2).
  - Block 4 (block-distance-4 pairs) is covered by BOTH endpoint cores, so
    each core only needs the ROW direction there: one DVE tensor_reduce
    (max) per span replaces exp + column sums.  Host combines
    max(LSE(blocks 0-3), blk4_rowmax).
  - Self block triangular at 128-col granularity.
  - blk4 spans are interleaved between LSE spans so PE/ACT/DVE overlap.

See kernel_v2.py docstring for the overall scheme (circulant cover, LSE).
"""

import sys

if "/opt/trn_rl_repo" not in sys.path:
    sys.path.insert(0, "/opt/trn_rl_repo")

import numpy as np

P = 128
D = 256
B_FULL = 16384
N_CORES = 8
BLK = 2048
NLSE = 4  # blocks 0..3 via LSE
NQ = 16

BETA = 768.0
BIAS = 227.0
SCALE_IN = 16.0
ACT_SCALE = BETA / (SCALE_IN * SCALE_IN)


def build_nc():
    import concourse.mybir as mybir
    import concourse.tile as tile
    from concourse import bacc
    from concourse.masks import make_identity

    dt = mybir.dt
    AF = mybir.ActivationFunctionType
    OP = mybir.AluOpType
    DR = mybir.MatmulPerfMode.DoubleRow

    nc = bacc.Bacc(None)
    xq_in = nc.declare_dram_parameter("xq", [P, 2, B_FULL], dt.float8e4, isOutput=False)
    ni_in = nc.declare_dram_parameter("negi", [P, P], dt.float8e4, isOutput=False)
    pi_in = nc.declare_dram_parameter("posi", [P, P], dt.float8e4, isOutput=False)
    srow_d = nc.declare_dram_parameter("srow", [P, NLSE * NQ], dt.float32, isOutput=True)
    # two column-sum accumulators per block: [0]=DVE chain, [1]=GPSIMD chain
    scol_d = nc.declare_dram_parameter(
        "scol", [P, NLSE, 2, BLK], dt.bfloat16, isOutput=True
    )
    rmax_d = nc.declare_dram_parameter("rmax", [P, NQ], dt.float32, isOutput=True)

    def emit_mms(ps, xT, qi, blk, c0, diag=None):
        """Gram matmuls for span (qi, blk) covering span cols [c0, 2048).

        diag=(negi, posi): fold a -57600*I correction into the 512-tile
        containing span cols [qi*128, qi*128+128) to kill self-matches.
        """
        lhs = xT[:, :, qi * P : (qi + 1) * P]
        c = c0
        while c < BLK:
            w = min(512 - (c % 512), BLK - c)
            col = blk * BLK + c
            has_diag = diag is not None and c <= qi * P < c + w
            nc.tensor.matmul(
                ps[:, c : c + w],
                lhs,
                xT[:, :, col : col + w],
                start=True,
                stop=not has_diag,
                perf_mode=DR,
            )
            if has_diag:
                nc.tensor.matmul(
                    ps[:, qi * P : (qi + 1) * P],
                    diag[0],
                    diag[1],
                    start=False,
                    stop=True,
                )
            c += w

    with tile.TileContext(nc) as tc:
        with (
            tc.tile_pool(name="persist", bufs=1) as persist,
            tc.tile_pool(name="ep", bufs=4) as ep,
            tc.tile_pool(name="sp", bufs=2) as sp,
            tc.tile_pool(name="ps", bufs=2, space="PSUM") as psp,
        ):
            xT = persist.tile([P, 2, B_FULL], dt.float8e4)
            negi = persist.tile([P, P], dt.float8e4)
            nc.sync.dma_start(out=negi, in_=ni_in[:, :])
            posi = persist.tile([P, P], dt.float8e4)
            nc.sync.dma_start(out=posi, in_=pi_in[:, :])
            srow_sb = persist.tile([P, NLSE * NQ], dt.float32)
            rmax_sb = persist.tile([P, NQ], dt.float32)
            biasap = persist.tile([P, 1], dt.float32)
            nc.vector.memset(biasap, -BIAS)
            ident = persist.tile([P, P], dt.float16)
            make_identity(nc, ident)
            # preload the exp table set while DMA streams
            scratch1 = persist.tile([P, 1], dt.float32)
            nc.scalar.activation(
                out=scratch1, in_=biasap, func=AF.Exp, bias=biasap, scale=0.0
            )

            # alternate two DMA queues so chunks stream concurrently and the
            # first chunk (block 0 -- processed first) lands asap
            for i, ch in enumerate(range(0, 5 * BLK, BLK // 2)):
                q = nc.gpsimd if i % 2 == 0 else nc.sync
                q.dma_start(
                    out=xT[:, :, ch : ch + BLK // 2],
                    in_=xq_in[:, :, ch : ch + BLK // 2],
                )

            wps = psp.tile([P, BLK], dt.float32, tag="ps", name="warm")
            for _ in range(32):
                nc.tensor.matmul(wps[:, :P], ident, ident, start=True, stop=True)

            def emit_blk4(qi):
                ps = psp.tile([P, BLK], dt.float32, tag="ps")
                emit_mms(ps, xT, qi, 4, 0)
                nc.vector.tensor_reduce(
                    rmax_sb[:, qi : qi + 1], ps, axis=mybir.AxisListType.X, op=OP.max
                )

            # Self block first (needs only DMA chunk 0), with qi descending so
            # the narrow triangular spans fill the DMA window.  Then blocks
            # 1..3 full width; blk4 row-max spans interleave one per three
            # LSE spans once its DMA chunk is resident.  A slice of the
            # column-sum accumulation goes to GPSIMD (own chain) to keep DVE
            # below ACT.
            GP_QI = ()
            it = 0
            for blk in (0, 1, 2, 3):
                sblk = sp.tile([P, BLK], dt.bfloat16, tag="s")
                sblk_g = None
                qis = range(NQ - 1, -1, -1) if blk == 0 else range(NQ)
                if blk == 0:
                    nc.vector.memset(sblk, 0.0)
                for qi in qis:
                    c0 = P * qi if blk == 0 else 0
                    ps = psp.tile([P, BLK], dt.float32, tag="ps")
                    emit_mms(ps, xT, qi, blk, c0, diag=(negi, posi) if blk == 0 else None)
                    e = ep.tile([P, BLK], dt.bfloat16, tag="e")
                    nc.scalar.activation(
                        out=e[:, c0:BLK],
                        in_=ps[:, c0:BLK],
                        func=AF.Exp,
                        bias=biasap,
                        scale=ACT_SCALE,
                        accum_out=srow_sb[:, blk * NQ + qi : blk * NQ + qi + 1],
                    )
                    if blk == 0:
                        nc.vector.tensor_tensor(
                            sblk[:, c0:BLK], e[:, c0:BLK], sblk[:, c0:BLK], OP.add
                        )
                    elif qi in GP_QI:
                        if sblk_g is None:
                            sblk_g = sp.tile([P, BLK], dt.bfloat16, tag="sg")
                            nc.gpsimd.tensor_copy(sblk_g, e)
                        else:
                            nc.gpsimd.tensor_tensor(sblk_g, e, sblk_g, OP.add)
                    elif qi == 0:
                        nc.vector.tensor_copy(sblk, e)
                    else:
                        nc.vector.tensor_tensor(sblk, e, sblk, OP.add)
                    if it >= 8 and (it - 8) % 3 == 0 and (it - 8) // 3 < NQ:
                        emit_blk4((it - 8) // 3)
                    it += 1
                nc.sync.dma_start(out=scol_d[:, blk, 0, :], in_=sblk)
                if sblk_g is not None:
                    nc.sync.dma_start(out=scol_d[:, blk, 1, :], in_=sblk_g)

            nc.sync.dma_start(out=srow_d[:, :], in_=srow_sb)
            nc.sync.dma_start(out=rmax_d[:, :], in_=rmax_sb)

    nc.compile()
    return nc


_NC_CACHE = {}


def _get_nc():
    if "nc" not in _NC_CACHE:
        _NC_CACHE["nc"] = build_nc()
    return _NC_CACHE["nc"]


LAST_RESULT = None


def kernel(student_output: np.ndarray) -> np.ndarray:
    import os

    import ml_dtypes
    from concourse.bass_utils import run_bass_kernel_spmd

    global LAST_RESULT
    x = np.ascontiguousarray(student_output, dtype=np.float32)
    assert x.shape == (B_FULL, D)

    norm = np.maximum(np.linalg.norm(x, axis=1, keepdims=True), 1e-12)
    xn = x / norm
    xq = (xn * SCALE_IN).astype(ml_dtypes.float8_e4m3)  # [B, 256]
    xT = np.ascontiguousarray(xq.T.reshape(2, P, B_FULL).transpose(1, 0, 2))

    ni = np.zeros((P, P), np.float32)
    np.fill_diagonal(ni, -240.0)
    pi = np.zeros((P, P), np.float32)
    np.fill_diagonal(pi, 240.0)
    ni = ni.astype(ml_dtypes.float8_e4m3)
    pi = pi.astype(ml_dtypes.float8_e4m3)

    nc = _get_nc()
    in_maps = [
        {"xq": np.roll(xT, -c * BLK, axis=2), "negi": ni, "posi": pi}
        for c in range(N_CORES)
    ]
    trace = bool(int(os.environ.get("KOLEO_TRACE", "0")))
    res = run_bass_kernel_spmd(
        nc, in_maps, core_ids=list(range(N_CORES)), trace=trace
    )
    LAST_RESULT = res

    S = np.zeros(B_FULL, np.float64)
    gmax4 = np.zeros(B_FULL, np.float64)
    for c in range(N_CORES):
        srow = np.asarray(res.results[c]["srow"], dtype=np.float64)  # [128, 64]
        scol = np.asarray(
            res.results[c]["scol"].astype(np.float32), dtype=np.float64
        )  # [128, 4, 2, 2048]
        rmax = np.asarray(res.results[c]["rmax"], dtype=np.float64)  # [128, 16]
        rp = srow.reshape(P, NLSE, NQ).sum(axis=1)  # [p, qi]
        rloc = rp.T.reshape(BLK)
        cloc = scol.sum(axis=(0, 2)).reshape(NLSE * BLK)
        mloc = rmax.T.reshape(BLK)  # local row qi*128+p
        base = c * BLK
        S[base : base + BLK] += rloc
        gmax4[base : base + BLK] = mloc / (SCALE_IN * SCALE_IN)
        for blk in range(NLSE):
            j0 = (base + blk * BLK) % B_FULL
            S[j0 : j0 + BLK] += cloc[blk * BLK : (blk + 1) * BLK]

    est = (np.log(S) + BIAS) / BETA
    g = np.maximum(est, gmax4)
    md = np.sqrt(np.clip(2.0 - 2.0 * g, 0.0, None))
    loss = -np.mean(np.log(md + 1e-8))
    return np.float32(loss)


if __name__ == "__main__":
    rng = np.random.default_rng(0)
    x = rng.standard_normal((B_FULL, D), dtype=np.float32)
    out = kernel(x)
    print("loss:", out)
